# revision 18
# baseline (speedup 1.0000x reference)
"""3-layer GAT on 8 Trainium2 NeuronCores (Bass/Tile SPMD), v3.

Sharding: nodes partitioned into 8 contiguous blocks of 6250 (padded to 6272);
edges assigned to the core owning their dst node, so per-dst softmax and
scatter-add stay local.

Per layer, per core:
 1. Node phase: [h|es|ed] = X @ [W | W@As | W@Ad] (one bf16 matmul per
    128-node tile). [h|es] rows go to DRAM as bf16 in a 512B-stride
    gather-ready layout; ed rows stay in SBUF ([128d, TILES, 4]).
    For layers 1/2 the node tiles are emitted interleaved with the previous
    layer's post groups (each group's transposes feed its node tiles), so
    the next AllGather starts while the previous layer's tail still runs.
 2. AllGather the [h|es] shards directly in the strided layout (no repack),
    in two halves A/B so edge processing of half A overlaps the AllGather
    of half B.
 3. Edge phase per 1024-edge slice: one dma_gather pulls [h|es] rows
    (264B payload each). Host-shipped fp8 one-hot matrices ride in with a
    single DMA per slice: OH[e,d] (acc-scatter lhsT) and OHDE[d,e]
    (ed-expand lhsT). Per chunk, edpe[:,c,:] = OHDE_c.T @ ed_blk puts
    ed[dst] in PSUM; z = es + ed; w = exp(leakyrelu(z)) is computed as
    max(exp(z), exp(0.2 z)) on the Scalar engine (exp is monotone).
    msg = h*w in bf16 (DVE for half A, GpSimd for half B — perf probe);
    one accumulating matmul per chunk scatters [w | msg] into the dst
    block's PSUM. Softmax max-subtraction is skipped (logits ~1e-1;
    mathematically identical).
 4. Post phase in groups of 7 blocks (amortizes DVE op overhead and keeps
    queues short): normalize, bias, ELU (= relu(x)+exp(min(x,0))-1 with -1
    absorbed by LayerNorm's shift invariance), LayerNorm (rsqrt via
    exp(-0.5*ln(v))), residual (layer 1), PE transposes for the next
    layer's bf16 lhsT. All ACT functions (exp/ln/square) live in the single
    pinned table set natural_log_exp_and_others, so the table loads once.

int16 gather indices limit sources to 32768 rows, so gather arrays are split
into halves by node local index (< 3200 vs >= 3200), each < 32768 rows.
"""

import inspect
import textwrap

import numpy as np
import ml_dtypes

import concourse.bass as bass
import concourse.mybir as mybir
import concourse.tile as tile
from concourse import bacc
from concourse.bass_utils import run_bass_kernel_spmd

# ---- problem constants (hardcoded; must match the grader's reference) ----
N, E = 50000, 800000
NEG = 0.2
EPS = 1e-5
NCORES = 8
NLOC = 6250           # real nodes per core
NSH = 6272            # padded nodes per core (49 tiles of 128)
TILES = NSH // 128    # 49
A_LOC = 3200          # locals < A_LOC -> half A (25 tiles)
B_LOC = NSH - A_LOC   # 3072 (24 tiles)
A_TILES = A_LOC // 128
NA = NCORES * A_LOC   # 25600 (< 32768, int16-safe)
NB = NCORES * B_LOC   # 24576
ROWF = 132            # gathered row: h(128) + es(4) bf16 values
STRIDEE = 256         # gather-array row stride in bf16 elems (512B)
SLICE_CH = 8          # chunks per gather instruction (1024 idx ring limit)
GSZ = 7               # post-phase group size (49 = 7*7 blocks)

f32 = mybir.dt.float32
bf16 = mybir.dt.bfloat16
fp8 = mybir.dt.float8e4
i16 = mybir.dt.int16

TRACE = False
LAST_EXEC_NS = None
LAST_RESULTS = None
_PROGRAM_CACHE = {}


def _patch_dma_gather():
    """Relax dma_gather's elem_size%256 assert (the firmware constraint is on
    the row *stride*, which stays 256B-aligned); enables 264B elements."""
    if getattr(bass.BassGpSimd.dma_gather, "_patched", False):
        return
    src = textwrap.dedent(inspect.getsource(bass.BassGpSimd.dma_gather))
    assert "elem_size_bytes % 256 == 0" in src
    src = src.replace(
        "elem_size_bytes > 0 and elem_size_bytes % 256 == 0",
        "elem_size_bytes > 0",
    )
    ns = vars(bass).copy()
    exec(compile(src, "<patched_dma_gather>", "exec"), ns)
    fn = ns["dma_gather"]
    fn._patched = True
    bass.BassGpSimd.dma_gather = fn


def _patch_act_tables():
    """Confine exp/ln/square to the one set that has all three
    (natural_log_exp_and_others) so the ACT table loads exactly once,
    instead of thrashing Exp<->Ln every LayerNorm."""
    if getattr(bacc, "_act_tables_pinned", False):
        return
    orig = bacc.get_activation_tables

    def pinned(arch):
        tabs = dict(orig(arch))  # preserves insertion order == set ids
        tgt = "natural_log_exp_and_others"
        if tgt in tabs:
            drop = {
                mybir.ActivationFunctionType.Exp,
                mybir.ActivationFunctionType.Ln,
                mybir.ActivationFunctionType.Square,
            }
            tabs = {
                name: (s if name == tgt else s - drop)
                for name, s in tabs.items()
            }
        return tabs

    bacc.get_activation_tables = pinned
    bacc._act_tables_pinned = True


def _install_ntff_hook():
    """Register the axon NTFF profiling hook (antenv.axon_hooks is missing in
    this image) so run_bass_kernel_spmd(trace=True) returns exec_time_ns."""
    import sys
    import types
    if "antenv.axon_hooks" in sys.modules:
        return
    import antenv
    mod = types.ModuleType("antenv.axon_hooks")
    _h = [None]
    mod.set_axon_ntff_profile_hook = lambda h: _h.__setitem__(0, h)
    mod.get_axon_ntff_profile_hook = lambda: _h[0]
    sys.modules["antenv.axon_hooks"] = mod
    antenv.axon_hooks = mod
    from trn_agent_boot.trn_boot import _ntff_profile_via_ctypes
    mod.set_axon_ntff_profile_hook(
        _ntff_profile_via_ctypes("/opt/axon/libaxon_pjrt.so"))


def _slices_of_half(cbh_list, half):
    """Static chunk layout for one half: blocks[j] = dst block of chunk j,
    first/last[j] = whether chunk j is the first/last of its block-half."""
    blocks = []
    first = []
    last = []
    for b in range(TILES):
        n = cbh_list[b][half]
        for j in range(n):
            blocks.append(b)
            first.append(j == 0)
            last.append(j == n - 1)
    return blocks, first, last


def _build_program(cbh_list):
    """Build the SPMD Bass program. cbh_list[b][h] = chunks (128 edges) for
    dst block b, source half h — identical across cores (SPMD)."""
    _patch_dma_gather()
    _patch_act_tables()
    nc = bacc.Bacc("TRN2", num_swdge_queues=4)
    ch_half = [sum(cbh_list[b][h] for b in range(TILES)) for h in range(2)]

    # ---- external inputs ----
    XT0 = nc.dram_tensor("XT0", [128, NSH], bf16, kind="ExternalInput")
    WEXT = nc.dram_tensor("WEXT", [3, 128, 136], bf16, kind="ExternalInput")
    WSK = nc.dram_tensor("WSK", [128, 128], bf16, kind="ExternalInput")
    BREP = nc.dram_tensor("BREP", [2, 128, 128], f32, kind="ExternalInput")
    GREP = nc.dram_tensor("GREP", [2, 128, 128], f32, kind="ExternalInput")
    BEREP = nc.dram_tensor("BEREP", [2, 128, 128], f32, kind="ExternalInput")
    B3REP = nc.dram_tensor("B3REP", [128, 32], f32, kind="ExternalInput")
    IDEN = nc.dram_tensor("IDEN", [128, 128], bf16, kind="ExternalInput")
    IDXH = [nc.dram_tensor(f"IDXH{h}", [128, ch_half[h] * 8], i16,
                           kind="ExternalInput") for h in range(2)]
    # [:,0,:] = OH[e,d] acc-scatter lhsT; [:,1,:] = OHDE[d,e] ed-expand lhsT
    OHD2 = [nc.dram_tensor(f"OHD2{h}", [128, 2, ch_half[h] * 128], fp8,
                           kind="ExternalInput") for h in range(2)]
    OUT = nc.dram_tensor("OUT", [NSH, 32], f32, kind="ExternalOutput")

    ts = bass.ts
    TT = mybir.AluOpType
    ACT = mybir.ActivationFunctionType

    with tile.TileContext(nc, num_cores=NCORES) as tc:
        with (
            tc.tile_pool(name="dram", bufs=1, space="DRAM") as dp,
            tc.tile_pool(name="consts", bufs=1) as cp,
            tc.tile_pool(name="xtp", bufs=2) as xtp,
            tc.tile_pool(name="node", bufs=3) as npo,
            tc.tile_pool(name="edsb", bufs=2) as edp_,
            tc.tile_pool(name="accsb", bufs=1) as asb,
            tc.tile_pool(name="totb", bufs=1) as tb_p,
            tc.tile_pool(name="idx", bufs=8) as ip,
            tc.tile_pool(name="gat", bufs=8) as gp,
            tc.tile_pool(name="ohp", bufs=7) as op_,
            tc.tile_pool(name="edge", bufs=4) as ep,
            tc.tile_pool(name="rhsp", bufs=4) as rp,
            tc.tile_pool(name="post", bufs=5) as qp,
            tc.tile_pool(name="tiny", bufs=6) as tp,
            # PSUM budget (8 banks): acc x3, node/trans/resid x2, edpe x3
            tc.tile_pool(name="accps", bufs=3, space="PSUM") as ap_p,
            tc.tile_pool(name="nodeps", bufs=2, space="PSUM") as np_p,
            tc.tile_pool(name="edpeps", bufs=3, space="PSUM") as ed_p,
        ):
            # ---- internal DRAM ----
            hx_sh = [dp.tile([A_LOC, STRIDEE], bf16, name="hx_shA"),
                     dp.tile([B_LOC, STRIDEE], bf16, name="hx_shB")]
            # Shared collective outputs must be single-writer: one per layer
            hc_l = [[dp.tile([NA, STRIDEE], bf16, name=f"hc_A{l}",
                             addr_space="Shared"),
                     dp.tile([NB, STRIDEE], bf16, name=f"hc_B{l}",
                             addr_space="Shared")] for l in range(3)]

            # ---- constants ----
            wext_t = cp.tile([128, 3, 136], bf16, name="wext_t")
            nc.sync.dma_start(wext_t[:], WEXT[:].rearrange("l p f -> p l f"))
            wsk_t = cp.tile([128, 128], bf16, name="wsk_t")
            nc.sync.dma_start(wsk_t[:], WSK[:])
            brep_t = cp.tile([128, 2, 128], f32, name="brep_t")
            nc.sync.dma_start(brep_t[:], BREP[:].rearrange("l p f -> p l f"))
            grep_t = cp.tile([128, 2, 128], f32, name="grep_t")
            nc.sync.dma_start(grep_t[:], GREP[:].rearrange("l p f -> p l f"))
            berep_t = cp.tile([128, 2, 128], f32, name="berep_t")
            nc.sync.dma_start(berep_t[:], BEREP[:].rearrange("l p f -> p l f"))
            b3_t = cp.tile([128, 32], f32, name="b3_t")
            nc.sync.dma_start(b3_t[:], B3REP[:])
            iden_t = cp.tile([128, 128], bf16, name="iden_t")
            nc.sync.dma_start(iden_t[:], IDEN[:])

            xt0_t = xtp.tile([128, NSH], bf16, tag="xt", name="xt0_t")
            nc.sync.dma_start(xt0_t[:], XT0[:])

            def node_tile(lay, t, xt_src, eds_dst):
                nps = np_p.tile([128, 136], f32, space="PSUM", tag="nps",
                                name="nps")
                nc.tensor.matmul(nps[:], lhsT=xt_src[:, ts(t, 128)],
                                 rhs=wext_t[:, lay, :], start=True,
                                 stop=True)
                nsb = npo.tile([128, ROWF], bf16, tag="nsb", name="nsb")
                nc.vector.tensor_copy(nsb[:], nps[:, 0:ROWF])
                nc.vector.tensor_copy(eds_dst[:, t, :], nps[:, 132:136])
                if t < A_TILES:
                    nc.sync.dma_start(hx_sh[0][ts(t, 128), 0:ROWF], nsb[:])
                else:
                    nc.sync.dma_start(
                        hx_sh[1][ts(t - A_TILES, 128), 0:ROWF], nsb[:])

            def allgather(lay, hf):
                nc.gpsimd.collective_compute(
                    "AllGather", mybir.AluOpType.bypass,
                    replica_groups=[list(range(NCORES))],
                    ins=[hx_sh[hf].opt()], outs=[hc_l[lay][hf].opt()])

            # layer 0 node phase
            eds_cur = edp_.tile([128, TILES, 4], bf16, tag="eds",
                                name="eds0")
            for t in range(TILES):
                node_tile(0, t, xt0_t, eds_cur)
                if t == A_TILES - 1:
                    allgather(0, 0)
            allgather(0, 1)

            xt_cur = xt0_t
            pending_tail = []
            for lay in range(3):
                xt_next = (xtp.tile([128, NSH], bf16, tag="xt",
                                    name=f"xt{lay + 1}_t")
                           if lay < 2 else None)
                eds_next = (edp_.tile([128, TILES, 4], bf16, tag="eds",
                                      name=f"eds{lay + 1}")
                            if lay < 2 else None)
                totbuf = tb_p.tile([128, TILES, ROWF], f32, tag="tot",
                                   name="totbuf")
                acc_a = asb.tile([128, TILES, ROWF], f32, tag="acc_a",
                                 name="acc_a")
                acc_ps = {}  # block -> live PSUM acc tile

                def emit_post_group(g, lay=lay, totbuf=totbuf,
                                    xt_next=xt_next, eds_next=eds_next,
                                    xt_cur=xt_cur):
                    b0 = g * GSZ
                    gs = min(GSZ, TILES - b0)
                    T = totbuf[:, b0:b0 + gs, :]
                    dn = tp.tile([128, GSZ, 4], f32, tag="dn", name="dn")
                    nc.vector.tensor_scalar(
                        out=dn[:, 0:gs, :], in0=T[:, :, 0:4], scalar1=1e-16,
                        scalar2=None, op0=TT.add)
                    rc = tp.tile([128, GSZ, 4], f32, tag="rc", name="rc")
                    nc.vector.reciprocal(rc[:, 0:gs, :], dn[:, 0:gs, :])
                    onorm = qp.tile([128, GSZ, 128], f32, tag="pg",
                                    name="onorm")
                    nc.vector.tensor_tensor(
                        out=onorm[:, 0:gs, :].rearrange(
                            "p b (h v) -> p b h v", h=4),
                        in0=T[:, :, 4:132].rearrange(
                            "p b (h v) -> p b h v", h=4),
                        in1=rc[:, 0:gs, :, None].to_broadcast(
                            [128, gs, 4, 32]),
                        op=TT.mult)
                    if lay == 2:
                        hm = qp.tile([128, GSZ, 32], f32, tag="po",
                                     name="hm")
                        nc.vector.tensor_reduce(
                            out=hm[:, 0:gs, :],
                            in_=onorm[:, 0:gs, :].rearrange(
                                "p b (h v) -> p b v h", h=4),
                            axis=mybir.AxisListType.X, op=TT.add)
                        ho = qp.tile([128, GSZ, 32], f32, tag="po",
                                     name="ho")
                        nc.vector.tensor_scalar(
                            out=ho[:, 0:gs, :], in0=hm[:, 0:gs, :],
                            scalar1=0.25, scalar2=None, op0=TT.mult)
                        hb = qp.tile([128, GSZ, 32], f32, tag="po",
                                     name="hb")
                        nc.vector.tensor_tensor(
                            out=hb[:, 0:gs, :], in0=ho[:, 0:gs, :],
                            in1=b3_t[:, None, :].to_broadcast([128, gs, 32]),
                            op=TT.add)
                        nc.sync.dma_start(
                            OUT[b0 * 128:(b0 + gs) * 128, :].rearrange(
                                "(b p) v -> p b v", p=128),
                            hb[:, 0:gs, :])
                        return
                    u0 = qp.tile([128, GSZ, 128], f32, tag="pg", name="u0")
                    nc.vector.tensor_tensor(
                        out=u0[:, 0:gs, :], in0=onorm[:, 0:gs, :],
                        in1=brep_t[:, lay, None, :].to_broadcast(
                            [128, gs, 128]),
                        op=TT.add)
                    mm_ = qp.tile([128, GSZ, 128], f32, tag="pg", name="mm_")
                    nc.vector.tensor_scalar(
                        out=mm_[:, 0:gs, :], in0=u0[:, 0:gs, :], scalar1=0.0,
                        scalar2=None, op0=TT.min)
                    em = qp.tile([128, GSZ, 128], f32, tag="pg", name="em")
                    nc.scalar.activation(em[:, 0:gs, :], mm_[:, 0:gs, :],
                                         ACT.Exp)
                    rl = qp.tile([128, GSZ, 128], f32, tag="pg", name="rl")
                    nc.vector.tensor_scalar(
                        out=rl[:, 0:gs, :], in0=u0[:, 0:gs, :], scalar1=0.0,
                        scalar2=None, op0=TT.max)
                    u = qp.tile([128, GSZ, 128], f32, tag="pg", name="u")
                    nc.vector.tensor_tensor(out=u[:, 0:gs, :],
                                            in0=em[:, 0:gs, :],
                                            in1=rl[:, 0:gs, :], op=TT.add)
                    ss = tp.tile([128, GSZ], f32, tag="ss", name="ss")
                    nc.vector.tensor_reduce(out=ss[:, 0:gs], in_=u[:, 0:gs, :],
                                            axis=mybir.AxisListType.X,
                                            op=TT.add)
                    mu = tp.tile([128, GSZ], f32, tag="mu", name="mu")
                    nc.vector.tensor_scalar(out=mu[:, 0:gs], in0=ss[:, 0:gs],
                                            scalar1=1.0 / 128, scalar2=None,
                                            op0=TT.mult)
                    xc = qp.tile([128, GSZ, 128], f32, tag="pg", name="xc")
                    nc.vector.tensor_tensor(
                        out=xc[:, 0:gs, :], in0=u[:, 0:gs, :],
                        in1=mu[:, 0:gs, None].to_broadcast([128, gs, 128]),
                        op=TT.subtract)
                    sq = qp.tile([128, GSZ, 128], f32, tag="pg", name="sq")
                    nc.scalar.activation(sq[:, 0:gs, :], xc[:, 0:gs, :],
                                         ACT.Square)
                    ssq = tp.tile([128, GSZ], f32, tag="ssq", name="ssq")
                    nc.vector.tensor_reduce(out=ssq[:, 0:gs],
                                            in_=sq[:, 0:gs, :],
                                            axis=mybir.AxisListType.X,
                                            op=TT.add)
                    var = tp.tile([128, GSZ], f32, tag="var", name="var")
                    nc.vector.tensor_scalar(out=var[:, 0:gs],
                                            in0=ssq[:, 0:gs],
                                            scalar1=1.0 / 128, scalar2=EPS,
                                            op0=TT.mult, op1=TT.add)
                    lnv = tp.tile([128, GSZ], f32, tag="lnv", name="lnv")
                    nc.scalar.activation(lnv[:, 0:gs], var[:, 0:gs], ACT.Ln)
                    rstd = tp.tile([128, GSZ], f32, tag="rstd", name="rstd")
                    nc.scalar.activation(rstd[:, 0:gs], lnv[:, 0:gs],
                                         ACT.Exp, scale=-0.5)
                    xn = qp.tile([128, GSZ, 128], f32, tag="pg", name="xn")
                    nc.vector.tensor_tensor(
                        out=xn[:, 0:gs, :], in0=xc[:, 0:gs, :],
                        in1=rstd[:, 0:gs, None].to_broadcast([128, gs, 128]),
                        op=TT.mult)
                    xg = qp.tile([128, GSZ, 128], f32, tag="pg", name="xg")
                    nc.vector.tensor_tensor(
                        out=xg[:, 0:gs, :], in0=xn[:, 0:gs, :],
                        in1=grep_t[:, lay, None, :].to_broadcast(
                            [128, gs, 128]),
                        op=TT.mult)
                    if lay == 0:
                        xb = qp.tile([128, GSZ, 128], f32, tag="pg",
                                     name="xb")
                        nc.vector.tensor_tensor(
                            out=xb[:, 0:gs, :], in0=xg[:, 0:gs, :],
                            in1=berep_t[:, lay, None, :].to_broadcast(
                                [128, gs, 128]),
                            op=TT.add)
                        for j in range(gs):
                            b = b0 + j
                            rps = np_p.tile([128, 128], f32, space="PSUM",
                                            tag="nps", name="rps")
                            nc.tensor.matmul(rps[:],
                                             lhsT=xt0_t[:, ts(b, 128)],
                                             rhs=wsk_t[:], start=True,
                                             stop=True)
                            xf = qp.tile([128, 128], bf16, tag="xf",
                                         name="xf")
                            nc.vector.tensor_tensor(out=xf[:],
                                                    in0=xb[:, j, :],
                                                    in1=rps[:], op=TT.add)
                            tps = np_p.tile([128, 128], bf16, space="PSUM",
                                            tag="nps", name="tps")
                            nc.tensor.transpose(tps[:], xf[:], iden_t[:])
                            nc.vector.tensor_copy(xt_next[:, ts(b, 128)],
                                                  tps[:])
                    else:
                        xfg = qp.tile([128, GSZ, 128], bf16, tag="xfg",
                                      name="xfg")
                        nc.vector.tensor_tensor(
                            out=xfg[:, 0:gs, :], in0=xg[:, 0:gs, :],
                            in1=berep_t[:, lay, None, :].to_broadcast(
                                [128, gs, 128]),
                            op=TT.add)
                        for j in range(gs):
                            b = b0 + j
                            tps = np_p.tile([128, 128], bf16, space="PSUM",
                                            tag="nps", name="tps")
                            nc.tensor.transpose(tps[:], xfg[:, j, :],
                                                iden_t[:])
                            nc.vector.tensor_copy(xt_next[:, ts(b, 128)],
                                                  tps[:])
                    # next layer's node tiles run on PE; defer them by one
                    # slice (pe_tail) so the post chain doesn't block the
                    # PE queue ahead of upcoming edpe/acc matmuls.
                    def pe_tail(b0=b0, gs=gs, lay=lay, xt_next=xt_next,
                                eds_next=eds_next):
                        for j in range(gs):
                            b = b0 + j
                            node_tile(lay + 1, b, xt_next, eds_next)
                        if b0 <= A_TILES - 1 < b0 + gs:
                            allgather(lay + 1, 0)
                        if b0 + gs == TILES:
                            allgather(lay + 1, 1)
                    pending_tail.append([2, pe_tail])

                halves = [_slices_of_half(cbh_list, hf) for hf in range(2)]
                slices = [(hf, s0)
                          for hf in range(2)
                          for s0 in range(0, len(halves[hf][0]), SLICE_CH)]
                stA = {}
                stE = {}

                def emit_A1(i, lay=lay):
                    hf, s0 = slices[i]
                    blocks = halves[hf][0]
                    sc = min(SLICE_CH, len(blocks) - s0)
                    ne = sc * 128
                    idxh_t = ip.tile([128, SLICE_CH * 8], i16, tag="idxh",
                                     name="idxh_t")
                    nc.sync.dma_start(idxh_t[:, 0:sc * 8],
                                      IDXH[hf][:, s0 * 8:(s0 + sc) * 8])
                    ohd_t = op_.tile([128, 2, SLICE_CH * 128], fp8,
                                     tag="ohd", name="ohd_t")
                    nc.scalar.dma_start(
                        ohd_t[:, :, 0:sc * 128],
                        OHD2[hf][:, :, s0 * 128:(s0 + sc) * 128])

                    hg = gp.tile([128, SLICE_CH, ROWF], bf16, tag="hg",
                                 name="hg")
                    nc.gpsimd.dma_gather(
                        hg[:, 0:sc, :], hc_l[lay][hf][:, 0:ROWF],
                        idxh_t[:, 0:sc * 8], ne, ne, ROWF,
                        elem_step=STRIDEE, queue_num=0)
                    stA[i] = (ohd_t, hg, sc)

                def emit_A2(i, eds_cur=eds_cur):
                    hf, s0 = slices[i]
                    blocks = halves[hf][0]
                    ohd_t, hg, sc = stA[i]
                    # ed[dst] per edge via one-hot matmuls into PSUM
                    edpe = ed_p.tile([128, SLICE_CH, 4], f32, space="PSUM",
                                     tag="edpe", name="edpe")
                    for c in range(sc):
                        nc.tensor.matmul(
                            edpe[:, c, :],
                            lhsT=ohd_t[:, 1, c * 128:(c + 1) * 128],
                            rhs=eds_cur[:, blocks[s0 + c], :],
                            start=True, stop=True, skip_group_check=True)
                    stE[i] = edpe

                def emit_B(i, acc_a=acc_a, totbuf=totbuf):
                    hf, s0 = slices[i]
                    blocks, first, last = halves[hf]
                    ohd_t, hg, sc = stA.pop(i)
                    edpe = stE.pop(i)
                    z = ep.tile([128, SLICE_CH, 4], bf16, tag="z", name="z")
                    nc.vector.tensor_tensor(
                        out=z[:, 0:sc, :], in0=hg[:, 0:sc, 128:132],
                        in1=edpe[:, 0:sc, :], op=TT.add)
                    # w = exp(leakyrelu(z)) = max(exp(z), exp(0.2 z))
                    e1 = ep.tile([128, SLICE_CH, 4], bf16, tag="e1",
                                 name="e1")
                    nc.scalar.activation(e1[:, 0:sc, :], z[:, 0:sc, :],
                                         ACT.Exp)
                    e2 = ep.tile([128, SLICE_CH, 4], bf16, tag="e2",
                                 name="e2")
                    nc.scalar.activation(e2[:, 0:sc, :], z[:, 0:sc, :],
                                         ACT.Exp, scale=NEG)
                    rhs_t = rp.tile([128, SLICE_CH, ROWF], bf16, tag="rhs",
                                    name="rhs_t")
                    nc.vector.tensor_tensor(
                        out=rhs_t[:, 0:sc, 0:4], in0=e1[:, 0:sc, :],
                        in1=e2[:, 0:sc, :], op=TT.max)
                    nc.vector.tensor_tensor(
                        out=rhs_t[:, 0:sc, 4:132].rearrange(
                            "p c (h v) -> p c h v", h=4),
                        in0=hg[:, 0:sc, 0:128].rearrange(
                            "p c (h v) -> p c h v", h=4),
                        in1=rhs_t[:, 0:sc, 0:4][:, :, :, None].to_broadcast(
                            [128, sc, 4, 32]),
                        op=TT.mult)
                    for c in range(sc):
                        j = s0 + c
                        b = blocks[j]
                        if first[j]:
                            acc_ps[b] = ap_p.tile(
                                [128, ROWF], f32, space="PSUM", tag="acc",
                                name="acc")
                        nc.tensor.matmul(
                            acc_ps[b][:],
                            lhsT=ohd_t[:, 0, c * 128:(c + 1) * 128],
                            rhs=rhs_t[:, c, :], start=first[j],
                            stop=last[j], skip_group_check=True)
                        if not last[j]:
                            continue
                        if hf == 0:
                            nc.vector.tensor_copy(acc_a[:, b, :],
                                                  acc_ps[b][:])
                            del acc_ps[b]
                            continue
                        nc.vector.tensor_tensor(
                            out=totbuf[:, b, :], in0=acc_a[:, b, :],
                            in1=acc_ps[b][:], op=TT.add)
                        del acc_ps[b]
                        if b % GSZ == GSZ - 1 or b == TILES - 1:
                            emit_post_group(b // GSZ)

                DEEP = 6
                for k in range(min(DEEP, len(slices))):
                    emit_A1(k)
                emit_A2(0)
                emit_A2(1)
                for i in range(len(slices)):
                    if i + DEEP < len(slices):
                        emit_A1(i + DEEP)
                    if i + 2 < len(slices):
                        emit_A2(i + 2)
                    for ent in pending_tail:
                        ent[0] -= 1
                    while pending_tail and pending_tail[0][0] <= 0:
                        pending_tail.pop(0)[1]()
                    emit_B(i)
                while pending_tail:
                    pending_tail.pop(0)[1]()
                if lay < 2:
                    xt_cur = xt_next
                    eds_cur = eds_next

    # Align gather queue_num with Tile's round-robin DMASW lane assignment
    # (lane i%8 <-> queue i%4 in scheduled Pool order) so each semaphore
    # lane is only ever used by a single SWDGE queue.
    gi = 0
    for bb in nc.main_func.blocks:
        for ins in bb.instructions:
            if isinstance(ins, mybir.InstDMAGatherAnt):
                ins.queue_num = gi % 4
                gi += 1
    nc.compile()
    return nc


def _wrap16(v):
    n = len(v)
    w = v.reshape(n // 16, 16).T  # [16, n/16]
    return np.tile(w, (8, 1)).astype(np.int16)


def _block_diag_att(a):
    # a: [4, C] -> [4*C, 4] block diagonal
    c = a.shape[1]
    out = np.zeros((4 * c, 4), np.float32)
    for h in range(4):
        out[h * c:(h + 1) * c, h] = a[h]
    return out


def prepare_inputs(x, edge_index, W1, as1, ad1, b1, g1, be1, W2, as2, ad2,
                   b2, g2, be2, W3, as3, ad3, b3, Wsk, bsk):
    x = np.asarray(x, np.float32)
    ei = np.asarray(edge_index)
    src = np.concatenate([ei[0], np.arange(N)]).astype(np.int64)
    dst = np.concatenate([ei[1], np.arange(N)]).astype(np.int64)

    score, sloc = src // NLOC, src % NLOC
    half = (sloc >= A_LOC).astype(np.int64)
    gidx = np.where(half == 0, A_LOC * score + sloc,
                    B_LOC * score + (sloc - A_LOC))
    dcore, dloc = dst // NLOC, dst % NLOC
    blk = dloc >> 7
    doff = dloc & 127

    # per-(block, half) chunk counts: max over cores (SPMD-identical program)
    gid = (dcore * TILES + blk) * 2 + half
    counts = np.bincount(gid, minlength=NCORES * TILES * 2).reshape(
        NCORES, TILES, 2)
    cbh = np.maximum(1, -(-counts.max(axis=0) // 128))  # [TILES, 2]
    cbh_list = tuple(tuple(int(v) for v in row) for row in cbh)

    # chunk offset of each (block, half) within its half's chunk sequence
    off = np.zeros((TILES, 2), np.int64)
    for h in range(2):
        off[:, h] = np.concatenate([[0], np.cumsum(cbh[:, h])[:-1]])
    ch_half = [int(cbh[:, h].sum()) for h in range(2)]

    in_maps = []
    wext = np.stack([
        np.concatenate([
            np.asarray(W, np.float32),
            np.asarray(W, np.float32) @ _block_diag_att(
                np.asarray(a_s, np.float32)),
            np.asarray(W, np.float32) @ _block_diag_att(
                np.asarray(a_d, np.float32))], axis=1)
        for (W, a_s, a_d) in [(W1, as1, ad1), (W2, as2, ad2), (W3, as3, ad3)]
    ]).astype(ml_dtypes.bfloat16)
    brep = np.stack([np.broadcast_to(np.asarray(b1, np.float32), (128, 128)),
                     np.broadcast_to(np.asarray(b2, np.float32), (128, 128))])
    grep = np.stack([np.broadcast_to(np.asarray(g1, np.float32), (128, 128)),
                     np.broadcast_to(np.asarray(g2, np.float32), (128, 128))])
    berep = np.stack([
        np.broadcast_to(np.asarray(be1, np.float32)
                        + np.asarray(bsk, np.float32), (128, 128)),
        np.broadcast_to(np.asarray(be2, np.float32), (128, 128))])
    b3rep = np.ascontiguousarray(
        np.broadcast_to(np.asarray(b3, np.float32), (128, 32)))
    iden = np.eye(128, dtype=np.float32).astype(ml_dtypes.bfloat16)
    wsk = np.asarray(Wsk, np.float32).astype(ml_dtypes.bfloat16)

    for c in range(NCORES):
        m = dcore == c
        e_blk, e_half, e_gidx, e_doff = blk[m], half[m], gidx[m], doff[m]
        core_map = {}
        for hf in range(2):
            mh = e_half == hf
            b_, g_, d_ = e_blk[mh], e_gidx[mh], e_doff[mh]
            order = np.lexsort((g_, b_))
            b_, g_, d_ = b_[order], g_[order], d_[order]
            starts = np.searchsorted(b_, np.arange(TILES))
            rank = np.arange(len(b_)) - starts[b_]
            pos = off[b_, hf] * 128 + rank
            ch = ch_half[hf]
            ne = ch * 128
            gi = np.zeros(ne, np.int64)
            gd = np.full(ne, -1, np.int64)
            gi[pos] = g_
            gd[pos] = d_
            core_map[f"IDXH{hf}"] = _wrap16(gi)
            M = np.zeros((ne, 128), np.float32)
            valid = np.nonzero(gd >= 0)[0]
            M[valid, gd[valid]] = 1.0
            M3 = M.reshape(ch, 128, 128)
            oh_p = M3.transpose(1, 0, 2).reshape(128, ch * 128)
            ohde_p = M3.transpose(2, 0, 1).reshape(128, ch * 128)
            core_map[f"OHD2{hf}"] = np.ascontiguousarray(
                np.stack([oh_p, ohde_p], axis=1)).astype(
                    ml_dtypes.float8_e4m3)
        xt = np.zeros((128, NSH), np.float32)
        xt[:, :NLOC] = x[c * NLOC:(c + 1) * NLOC].T
        core_map.update({
            "XT0": xt.astype(ml_dtypes.bfloat16), "WEXT": wext, "WSK": wsk,
            "BREP": brep, "GREP": grep, "BEREP": berep, "B3REP": b3rep,
            "IDEN": iden,
        })
        in_maps.append(core_map)

    return in_maps, cbh_list


def kernel(**inputs):
    global LAST_EXEC_NS, LAST_RESULTS
    in_maps, cbh_list = prepare_inputs(**inputs)
    if cbh_list not in _PROGRAM_CACHE:
        _PROGRAM_CACHE[cbh_list] = _build_program(cbh_list)
    nc = _PROGRAM_CACHE[cbh_list]

    if TRACE:
        _install_ntff_hook()
    res = run_bass_kernel_spmd(nc, in_maps, list(range(NCORES)),
                               trace=TRACE)
    LAST_EXEC_NS = res.exec_time_ns
    LAST_RESULTS = res
    out = np.concatenate(
        [res.results[c]["OUT"][:NLOC] for c in range(NCORES)], axis=0)
    return out.astype(np.float32)
